# revision 29
# baseline (speedup 1.0000x reference)
"""Trainium2 Bass kernel for nn_Attention_Module (sparse_attention).

Computation per batch b (x_b: [C=256, T=4096] fp32):
    energy = x_b @ x_b^T                      # (256, 256), K=4096
    attn   = softmax(rowmax(energy) - energy) # == exp(mu - e)/Z, mu = rowmin
    out    = gamma * (attn @ x_b) + x_b

Strategy (8 cores, pure data-parallel, 4 batches/core):
  - All HBM I/O in fp16: xT (for the energy matmul), x natural (for the
    second matmul + residual), and the output. 24 MB/core total vs 40 MB
    for the fp32 version; the kernel is DMA-bandwidth-bound (~330 GB/s/core)
    so bytes ~= time. fp16 energy passes the 2e-2 tolerance with margin
    (CPU sim: 7.6e-3).
  - fp16 weights also halve LDWEIGHTS time; matmul1 was ldweights-bound.
  - The +x residual is folded into matmul2 via the modified attention matrix
    A'' = gamma*P^T + diag(Z); out = diag(1/Z) * (A''^T @ x). Using the same
    (fp16-rounded) Z in diag and the post-scale keeps the x term exact.
  - matmul2 runs weight-runs of 4 (k outer, chunk inner over 4 PSUM banks)
    so walrus elides 3 of every 4 embedded weight loads.
  - Loads ride the sync-queue HWDGE ring; stores ride the gpsimd SWDGE
    queue, so store triggers waiting on compute never block load issue.
  - First batch's xT load is split into 4 chunks so matmul1 starts ~1.5us
    after the preamble; last batch's stores are split fine so the final
    drain is short.
"""

import numpy as np

B, C, T = 32, 256, 4096
NCORES = 8
NB = B // NCORES  # batches per core
P = 128
KT = T // P  # 32 k-tiles for the energy matmul
TC = T // 512  # 8 t-chunks for matmul2

_CACHE = {}


def _build_nc(variant=None):
    variant = variant or {}
    from contextlib import ExitStack

    import concourse.bacc as bacc
    import concourse.bass as bass
    import concourse.tile as tile
    from concourse import mybir

    f32 = mybir.dt.float32
    f16 = mybir.dt.float16
    xt_dt = mybir.dt.float32r if variant.get("xt_f32") else f16
    out_dt = f32 if variant.get("out_f32") else f16
    ts = bass.ts

    nc = bacc.Bacc(
        "TRN2",
        target_bir_lowering=False,
        debug=False,
        enable_asserts=False,
        num_devices=NCORES,
    )

    xt_h = nc.dram_tensor("xt", [NB, T, C], xt_dt, kind="ExternalInput")
    xn_h = nc.dram_tensor("xn", [NB, C, T], f16, kind="ExternalInput")
    # aux: per-partition row [gamma, 1/gamma, pad, pad, identity-row(128)]
    aux_h = nc.dram_tensor("aux", [P, 132], f32, kind="ExternalInput")
    o_h = nc.dram_tensor("o", [NB, C, T], out_dt, kind="ExternalOutput")

    store_q = variant.get("store_q", "act")
    mm2_run = variant.get("mm2_run", 4)  # weight-run length in matmul2

    with tile.TileContext(nc) as tc:
        with ExitStack() as ctx:
            singles = ctx.enter_context(tc.tile_pool(name="singles", bufs=1))
            xq_pool = ctx.enter_context(tc.tile_pool(name="xq", bufs=1))
            xt_pool = ctx.enter_context(tc.tile_pool(name="xt", bufs=4))
            xn_pool = ctx.enter_context(tc.tile_pool(name="xn", bufs=4))
            out_pool = ctx.enter_context(tc.tile_pool(name="out", bufs=4))
            att_pool = ctx.enter_context(tc.tile_pool(name="att", bufs=3))
            small = ctx.enter_context(tc.tile_pool(name="small", bufs=4))
            psum_e = ctx.enter_context(
                tc.tile_pool(name="psum_e", bufs=2, space="PSUM")
            )
            psum_t = ctx.enter_context(
                tc.tile_pool(name="psum_t", bufs=2, space="PSUM")
            )
            psum_o = ctx.enter_context(
                tc.tile_pool(
                    name="psum_o",
                    bufs=variant.get("psum_o", 5),
                    space="PSUM",
                )
            )

            xt_ap = xt_h.ap()
            xn_ap = xn_h.ap()
            o_ap = o_h.ap()

            store_eng = {
                "gpsimd": nc.gpsimd,
                "act": nc.scalar,
                "sync": nc.sync,
            }[store_q]

            # aux on the ACT ring so it doesn't delay the first xt load
            aux = singles.tile([P, 132], f32)
            nc.scalar.dma_start(aux[:], aux_h.ap())
            gv = aux[:, 0:1]
            ident = aux[:, 4:132]

            def issue_loads(b):
                KH = KT // 2
                # p-major tiling: partition p holds t in [p*KT, (p+1)*KT),
                # so each partition's source rows are contiguous in HBM
                # (8 KB descriptors instead of 512 B -> ~2x DMA efficiency).
                # The energy matmul contracts over all (p, k) pairs, so any
                # consistent permutation of t across lhsT/rhs is fine.
                xt_r = xt_ap[b].rearrange("(p k) c -> p k c", p=P)
                if b == 0:
                    # eight separate tiles so matmul1 starts after the first
                    # chunk lands (DMA-completion granularity)
                    KQ = KT // 8
                    qs = []
                    for q in range(8):
                        t_ = xq_pool.tile(
                            [P, KQ, C], xt_dt, tag=f"xq{q}", name=f"xq{q}"
                        )
                        nc.sync.dma_start(
                            t_[:], xt_r[:, q * KQ : (q + 1) * KQ, :]
                        )
                        qs.append(t_)
                    xt_tiles, kdiv = qs, KQ
                else:
                    xta = xt_pool.tile([P, KH, C], xt_dt, tag="xta", name="xta")
                    xtb = xt_pool.tile([P, KH, C], xt_dt, tag="xtb", name="xtb")
                    nc.sync.dma_start(xta[:], xt_r[:, :KH, :])
                    nc.sync.dma_start(xtb[:], xt_r[:, KH:, :])
                    xt_tiles, kdiv = [xta, xtb], KH
                return xt_tiles, kdiv

            def issue_xn(b):
                # xn(b) is first needed by matmul2(b), one iteration later
                # than xt(b) -- issuing it here keeps xt(b+1) ahead of it
                # in the sync-ring queue.
                xn = xn_pool.tile([P, 2, T], f16, tag="xn", name="xn")
                nc.sync.dma_start(
                    xn[:], xn_ap[b].rearrange("(m p) t -> p m t", p=P)
                )
                return xn

            xn_defer = variant.get("xn_defer", True)
            tiles = {0: issue_loads(0)}
            if not xn_defer:
                xns = {0: issue_xn(0)}
            pending = None  # (b, At, rZ, xn) awaiting matmul2

            for b in range(NB):
                xt, kdiv = tiles.pop(b)
                if b + 1 < NB:
                    tiles[b + 1] = issue_loads(b + 1)
                    if not xn_defer:
                        xns[b + 1] = issue_xn(b + 1)
                if not xn_defer:
                    xn = xns.pop(b)
                if variant.get("xn_defer", True):
                    xn = issue_xn(b)

                # A''^T, laid out [128(j within k-block), k-block, 256(i)]
                At = att_pool.tile([P, 2, C], f16)
                Zs = small.tile([P, 2], f32)
                Zb = small.tile([P, 2], f16)
                rZ = small.tile([P, 2], f32)

                # energy is symmetric: compute the full (0, :) block row and
                # the (1,1) diagonal block; mirror (1,0) = (0,1)^T via one
                # PE transpose. Saves 25% of matmul1 columns. pe0/pe1 share
                # one PSUM bank ([P, 384] slot).
                if variant.get("pt10_pack", True):
                    pef = psum_e.tile(
                        [P, 512], mybir.dt.float32, tag="pe", bufs=1
                    )
                    pt10 = pef[:, C + P : C + 2 * P]
                else:
                    pef = psum_e.tile(
                        [P, 384], mybir.dt.float32, tag="pe", bufs=1
                    )
                    pt10 = psum_t.tile(
                        [P, P], mybir.dt.float32, name="pt10",
                        tag="pt10", bufs=1,
                    )[:]
                pe0 = pef[:, 0:C]
                pe1 = pef[:, C : C + P]
                for k in range(KT):
                    src_t = xt[k // kdiv]
                    kk = k % kdiv
                    nc.tensor.matmul(
                        pe0,
                        lhsT=src_t[:, kk, ts(0, P)],
                        rhs=src_t[:, kk, :],
                        start=(k == 0),
                        stop=(k == KT - 1),
                    )
                for k in range(KT):
                    src_t = xt[k // kdiv]
                    kk = k % kdiv
                    nc.tensor.matmul(
                        pe1,
                        lhsT=src_t[:, kk, ts(1, P)],
                        rhs=src_t[:, kk, ts(1, P)],
                        start=(k == 0),
                        stop=(k == KT - 1),
                    )
                # raw energy block (0,1) -> SBUF so the PE can mirror it
                se01 = small.tile([P, P], f32, tag="se01")
                nc.vector.tensor_copy(se01[:], pe0[:, ts(1, P)])
                nc.tensor.transpose(pt10, se01[:], ident)

                mu0 = small.tile([P, 1], f32, tag="mu0")
                nc.vector.tensor_reduce(
                    mu0[:], pe0, axis=mybir.AxisListType.X,
                    op=mybir.AluOpType.min,
                )
                mu1a = small.tile([P, 2], f32, tag="mu1a")
                nc.vector.tensor_reduce(
                    mu1a[:, 0:1], pt10, axis=mybir.AxisListType.X,
                    op=mybir.AluOpType.min,
                )
                nc.vector.tensor_reduce(
                    mu1a[:, 1:2], pe1, axis=mybir.AxisListType.X,
                    op=mybir.AluOpType.min,
                )
                mu1 = small.tile([P, 1], f32, tag="mu1")
                nc.vector.tensor_reduce(
                    mu1[:], mu1a[:], axis=mybir.AxisListType.X,
                    op=mybir.AluOpType.min,
                )

                Pm0 = small.tile([P, C], f32, tag="Pm0")
                nc.scalar.activation(
                    Pm0[:], pe0, mybir.ActivationFunctionType.Exp,
                    bias=mu0[:], scale=-1.0, accum_out=Zs[:, 0:1],
                )
                Pm1 = small.tile([P, C], f32, tag="Pm1")
                Zpart = small.tile([P, 2], f32, tag="Zpart")
                nc.scalar.activation(
                    Pm1[:, 0:P], pt10, mybir.ActivationFunctionType.Exp,
                    bias=mu1[:], scale=-1.0, accum_out=Zpart[:, 0:1],
                )
                nc.scalar.activation(
                    Pm1[:, P:C], pe1, mybir.ActivationFunctionType.Exp,
                    bias=mu1[:], scale=-1.0, accum_out=Zpart[:, 1:2],
                )
                nc.vector.tensor_add(
                    Zs[:, 1:2], Zpart[:, 0:1], Zpart[:, 1:2]
                )
                nc.vector.tensor_copy(Zb[:], Zs[:])
                nc.vector.reciprocal(rZ[:], Zb[:])
                for m, Pm in ((0, Pm0), (1, Pm1)):
                    for k in range(2):
                        pt = psum_t.tile(
                            [P, P], mybir.dt.float32, name="pt", tag="pt"
                        )
                        nc.tensor.transpose(pt[:], Pm[:, ts(k, P)], ident)
                        # A''T[j in k-block, i in m-block] = gamma * P^T
                        # (on DVE: cheaper psum->sbuf than ACT, and keeps
                        # the ACT queue free for exps + scales)
                        if variant.get("mul_dve", True):
                            nc.vector.tensor_scalar_mul(
                                At[:, k, ts(m, P)], pt[:], gv
                            )
                        else:
                            nc.scalar.mul(At[:, k, ts(m, P)], pt[:], gv)
                    # diagonal: += diag(Z) (falls in the k == m block)
                    dg = small.tile([P, P], f16, tag="diag")
                    nc.vector.tensor_scalar_mul(dg[:], ident, Zs[:, m : m + 1])
                    nc.vector.tensor_add(
                        At[:, m, ts(m, P)], At[:, m, ts(m, P)], dg[:]
                    )

                # software-pipeline the PE: run the PREVIOUS batch's matmul2
                # after this batch's matmul1, hiding the A'' build latency.
                this = (b, At, rZ, xn)
                todo = [pending] if pending is not None else []
                if b == NB - 1:
                    todo.append(this)
                    pending = None
                else:
                    pending = this
                for pb, pAt, prZ, pxn in todo:
                    last = pb == NB - 1
                    for m in range(2):
                        for g in range(TC // mm2_run):
                            # ot staged per group: store as soon as the
                            # group's chunks are scaled
                            ot = out_pool.tile(
                                [P, 512 * mm2_run], out_dt, tag="ot", name="ot"
                            )
                            pos = [
                                psum_o.tile(
                                    [P, 512], mybir.dt.float32,
                                    name=f"po{j}", tag="po",
                                )
                                for j in range(mm2_run)
                            ]
                            # k outer, chunk inner: mm2_run consecutive
                            # matmuls share lhsT -> walrus elides the
                            # redundant embedded weight loads
                            for k in range(2):
                                for j in range(mm2_run):
                                    nc.tensor.matmul(
                                        pos[j][:],
                                        lhsT=pAt[:, k, ts(m, P)],
                                        rhs=pxn[
                                            :, k,
                                            ts(g * mm2_run + j, 512),
                                        ],
                                        start=(k == 0),
                                        stop=(k == 1),
                                    )
                            for j in range(mm2_run):
                                # out = psum * (1/Z); alternate engines so a
                                # group's scales finish before the next
                                # group's matmuls need the banks back
                                if j % 2 == 0:
                                    nc.vector.tensor_scalar_mul(
                                        ot[:, ts(j, 512)], pos[j][:],
                                        prZ[:, m : m + 1],
                                    )
                                else:
                                    nc.scalar.mul(
                                        ot[:, ts(j, 512)], pos[j][:],
                                        prZ[:, m : m + 1],
                                    )
                            gw = 512 * mm2_run
                            o_r = o_ap[pb].rearrange(
                                "(m p) t -> p m t", p=P
                            )
                            # last batch: loads are done, so the sync HWDGE
                            # ring is idle and much faster than SWDGE
                            seng = nc.sync if last else store_eng
                            nsub = 2 if last else 1
                            for sh in range(nsub):
                                sw = gw // nsub
                                seng.dma_start(
                                    o_r[:, m, g * gw + sh * sw : g * gw + (sh + 1) * sw],
                                    ot[:, sh * sw : (sh + 1) * sw],
                                )

    nc.compile()
    return nc


def _get_nc(variant=None):
    key = tuple(sorted((variant or {}).items()))
    if key not in _CACHE:
        _CACHE[key] = _build_nc(variant)
    return _CACHE[key]


def _make_aux(gamma_val):
    aux = np.zeros((P, 132), dtype=np.float32)
    aux[:, 0] = gamma_val
    aux[:, 1] = 1.0 / gamma_val if gamma_val != 0 else 0.0
    aux[:, 4:132] = np.eye(P, dtype=np.float32)
    return aux


def kernel(x, gamma, _trace=False, _variant=None):
    import concourse.bass_utils as bass_utils

    variant = _variant or {}
    x = np.ascontiguousarray(np.asarray(x, dtype=np.float32))
    gamma = np.asarray(gamma, dtype=np.float32).reshape(-1)

    nc = _get_nc(variant)

    xt_np_dt = np.float32 if variant.get("xt_f32") else np.float16
    aux = _make_aux(gamma[0])
    in_maps = []
    for d in range(NCORES):
        xs = x[d * NB : (d + 1) * NB]
        in_maps.append(
            {
                "xt": np.ascontiguousarray(
                    xs.transpose(0, 2, 1).astype(xt_np_dt)
                ),
                "xn": xs.astype(np.float16),
                "aux": aux,
            }
        )

    res = bass_utils.run_bass_kernel_spmd(
        nc, in_maps, core_ids=list(range(NCORES)), trace=_trace
    )
    out = np.concatenate(
        [r["o"].astype(np.float32) for r in res.results], axis=0
    )
    if _trace:
        _CACHE["last_results"] = res
    return out


# revision 32
# speedup vs baseline: 1.1056x; 1.1056x over previous
"""Trainium2 Bass kernel for nn_Attention_Module (sparse_attention).

Computation per batch b (x_b: [C=256, T=4096] fp32):
    energy = x_b @ x_b^T                      # (256, 256), K=4096
    attn   = softmax(rowmax(energy) - energy) # == exp(mu - e)/Z, mu = rowmin
    out    = gamma * (attn @ x_b) + x_b

Strategy (8 cores, pure data-parallel, 4 batches/core):
  - All HBM I/O in fp16: xT (for the energy matmul), x natural (for the
    second matmul + residual), and the output. 24 MB/core vs 40 MB for the
    fp32 version; the kernel was DMA-bandwidth-bound (~330 GB/s/core) so
    bytes ~= time. fp16 energy passes the 2e-2 tolerance with margin
    (measured 7.5e-3). fp16 weights also halve LDWEIGHTS time.
  - xT tiles are p-major (partition p holds t in [p*KT, (p+1)*KT)) so DMA
    descriptors are 8 KB contiguous instead of 512 B; the energy matmul is
    invariant to any consistent t-permutation of its operands.
  - Energy is symmetric: compute the (0,:) block row + (1,1) diagonal
    block, mirror (1,0) = (0,1)^T with one PE transpose (saves 25% of
    matmul1 columns). pe0/pe1/pt10 share one PSUM bank.
  - The +x residual is folded into matmul2 via the modified attention matrix
    A'' = gamma*P^T + diag(Z); out = diag(1/Z) * (A''^T @ x). Using the same
    (fp16-rounded) Z in diag and the post-scale keeps the x term exact.
  - matmul2 runs weight-runs of 4 (k outer, chunk inner over PSUM banks,
    5 banks in flight); psum->out scales alternate DVE/ACT so bank
    turnaround keeps pace with the PE.
  - Loads ride the sync HWDGE ring (xn deferred one iteration so next-batch
    xT jumps the queue); stores ride the ACT HWDGE ring so store triggers
    never block load issue; last-batch stores go back on the then-idle sync
    ring, split per 512-chunk to shorten the final drain.
  - First batch's xT load is split into 8 chunks so matmul1 starts right
    after the framework preamble.
"""

import numpy as np

B, C, T = 32, 256, 4096
NCORES = 8
NB = B // NCORES  # batches per core
P = 128
KT = T // P  # 32 k-tiles for the energy matmul
TC = T // 512  # 8 t-chunks for matmul2

_CACHE = {}


def _build_nc(variant=None):
    variant = variant or {}
    from contextlib import ExitStack

    import concourse.bacc as bacc
    import concourse.bass as bass
    import concourse.tile as tile
    from concourse import mybir

    f32 = mybir.dt.float32
    f16 = mybir.dt.float16
    xt_dt = mybir.dt.float32r if variant.get("xt_f32") else f16
    out_dt = f32 if variant.get("out_f32") else f16
    ts = bass.ts

    nc = bacc.Bacc(
        "TRN2",
        target_bir_lowering=False,
        debug=False,
        enable_asserts=False,
        num_devices=NCORES,
    )

    xt_h = nc.dram_tensor("xt", [NB, T, C], xt_dt, kind="ExternalInput")
    xn_h = nc.dram_tensor("xn", [NB, C, T], f16, kind="ExternalInput")
    # aux: per-partition row [gamma, 1/gamma, pad, pad, identity-row(128)]
    aux_h = nc.dram_tensor("aux", [P, 132], f32, kind="ExternalInput")
    o_h = nc.dram_tensor("o", [NB, C, T], out_dt, kind="ExternalOutput")

    store_q = variant.get("store_q", "act")
    mm2_run = variant.get("mm2_run", 4)  # weight-run length in matmul2

    with tile.TileContext(nc) as tc:
        with ExitStack() as ctx:
            singles = ctx.enter_context(tc.tile_pool(name="singles", bufs=1))
            xq_pool = ctx.enter_context(tc.tile_pool(name="xq", bufs=1))
            xt_pool = ctx.enter_context(tc.tile_pool(name="xt", bufs=4))
            xn_pool = ctx.enter_context(tc.tile_pool(name="xn", bufs=4))
            out_pool = ctx.enter_context(tc.tile_pool(name="out", bufs=4))
            att_pool = ctx.enter_context(tc.tile_pool(name="att", bufs=3))
            small = ctx.enter_context(tc.tile_pool(name="small", bufs=4))
            psum_e = ctx.enter_context(
                tc.tile_pool(name="psum_e", bufs=2, space="PSUM")
            )
            psum_t = ctx.enter_context(
                tc.tile_pool(name="psum_t", bufs=2, space="PSUM")
            )
            psum_o = ctx.enter_context(
                tc.tile_pool(
                    name="psum_o",
                    bufs=variant.get("psum_o", 5),
                    space="PSUM",
                )
            )

            xt_ap = xt_h.ap()
            xn_ap = xn_h.ap()
            o_ap = o_h.ap()

            store_eng = {
                "gpsimd": nc.gpsimd,
                "act": nc.scalar,
                "sync": nc.sync,
            }[store_q]

            # aux on the ACT ring so it doesn't delay the first xt load
            aux = singles.tile([P, 132], f32)
            nc.scalar.dma_start(aux[:], aux_h.ap())
            gv = aux[:, 0:1]
            ident = aux[:, 4:132]

            def issue_loads(b):
                KH = KT // 2
                # p-major tiling: partition p holds t in [p*KT, (p+1)*KT),
                # so each partition's source rows are contiguous in HBM
                # (8 KB descriptors instead of 512 B -> ~2x DMA efficiency).
                # The energy matmul contracts over all (p, k) pairs, so any
                # consistent permutation of t across lhsT/rhs is fine.
                xt_r = xt_ap[b].rearrange("(p k) c -> p k c", p=P)
                if b == 0:
                    # eight separate tiles so matmul1 starts after the first
                    # chunk lands (DMA-completion granularity)
                    KQ = KT // 8
                    qs = []
                    for q in range(8):
                        t_ = xq_pool.tile(
                            [P, KQ, C], xt_dt, tag=f"xq{q}", name=f"xq{q}"
                        )
                        nc.sync.dma_start(
                            t_[:], xt_r[:, q * KQ : (q + 1) * KQ, :]
                        )
                        qs.append(t_)
                    xt_tiles, kdiv = qs, KQ
                else:
                    xta = xt_pool.tile([P, KH, C], xt_dt, tag="xta", name="xta")
                    xtb = xt_pool.tile([P, KH, C], xt_dt, tag="xtb", name="xtb")
                    nc.sync.dma_start(xta[:], xt_r[:, :KH, :])
                    nc.sync.dma_start(xtb[:], xt_r[:, KH:, :])
                    xt_tiles, kdiv = [xta, xtb], KH
                return xt_tiles, kdiv

            def issue_xn(b):
                # xn(b) is first needed by matmul2(b), one iteration later
                # than xt(b) -- issuing it here keeps xt(b+1) ahead of it
                # in the sync-ring queue.
                xn = xn_pool.tile([P, 2, T], f16, tag="xn", name="xn")
                nc.sync.dma_start(
                    xn[:], xn_ap[b].rearrange("(m p) t -> p m t", p=P)
                )
                return xn

            xn_defer = variant.get("xn_defer", True)
            tiles = {0: issue_loads(0)}
            if not xn_defer:
                xns = {0: issue_xn(0)}
            pending = None  # (b, At, rZ, xn) awaiting matmul2

            for b in range(NB):
                xt, kdiv = tiles.pop(b)
                if b + 1 < NB:
                    tiles[b + 1] = issue_loads(b + 1)
                    if not xn_defer:
                        xns[b + 1] = issue_xn(b + 1)
                if not xn_defer:
                    xn = xns.pop(b)
                if variant.get("xn_defer", True):
                    xn = issue_xn(b)

                # A''^T, laid out [128(j within k-block), k-block, 256(i)]
                At = att_pool.tile([P, 2, C], f16)
                Zs = small.tile([P, 2], f32)
                Zb = small.tile([P, 2], f16)
                rZ = small.tile([P, 2], f32)

                # energy is symmetric: compute the full (0, :) block row and
                # the (1,1) diagonal block; mirror (1,0) = (0,1)^T via one
                # PE transpose. Saves 25% of matmul1 columns. pe0/pe1 share
                # one PSUM bank ([P, 384] slot).
                if variant.get("pt10_pack", True):
                    pef = psum_e.tile(
                        [P, 512], mybir.dt.float32, tag="pe",
                        bufs=variant.get("pe_bufs", 1),
                    )
                    pt10 = pef[:, C + P : C + 2 * P]
                else:
                    pef = psum_e.tile(
                        [P, 384], mybir.dt.float32, tag="pe", bufs=1
                    )
                    pt10 = psum_t.tile(
                        [P, P], mybir.dt.float32, name="pt10",
                        tag="pt10", bufs=1,
                    )[:]
                pe0 = pef[:, 0:C]
                pe1 = pef[:, C : C + P]
                for k in range(KT):
                    src_t = xt[k // kdiv]
                    kk = k % kdiv
                    nc.tensor.matmul(
                        pe0,
                        lhsT=src_t[:, kk, ts(0, P)],
                        rhs=src_t[:, kk, :],
                        start=(k == 0),
                        stop=(k == KT - 1),
                    )
                for k in range(KT):
                    src_t = xt[k // kdiv]
                    kk = k % kdiv
                    nc.tensor.matmul(
                        pe1,
                        lhsT=src_t[:, kk, ts(1, P)],
                        rhs=src_t[:, kk, ts(1, P)],
                        start=(k == 0),
                        stop=(k == KT - 1),
                    )
                # raw energy block (0,1) -> SBUF so the PE can mirror it
                se01 = small.tile([P, P], f32, tag="se01")
                nc.vector.tensor_copy(se01[:], pe0[:, ts(1, P)])
                nc.tensor.transpose(pt10, se01[:], ident)

                mu0 = small.tile([P, 1], f32, tag="mu0")
                nc.vector.tensor_reduce(
                    mu0[:], pe0, axis=mybir.AxisListType.X,
                    op=mybir.AluOpType.min,
                )
                mu1a = small.tile([P, 2], f32, tag="mu1a")
                nc.vector.tensor_reduce(
                    mu1a[:, 0:1], pt10, axis=mybir.AxisListType.X,
                    op=mybir.AluOpType.min,
                )
                nc.vector.tensor_reduce(
                    mu1a[:, 1:2], pe1, axis=mybir.AxisListType.X,
                    op=mybir.AluOpType.min,
                )
                mu1 = small.tile([P, 1], f32, tag="mu1")
                nc.vector.tensor_reduce(
                    mu1[:], mu1a[:], axis=mybir.AxisListType.X,
                    op=mybir.AluOpType.min,
                )

                Pm0 = small.tile([P, C], f32, tag="Pm0")
                nc.scalar.activation(
                    Pm0[:], pe0, mybir.ActivationFunctionType.Exp,
                    bias=mu0[:], scale=-1.0, accum_out=Zs[:, 0:1],
                )
                Pm1 = small.tile([P, C], f32, tag="Pm1")
                Zpart = small.tile([P, 2], f32, tag="Zpart")
                nc.scalar.activation(
                    Pm1[:, 0:P], pt10, mybir.ActivationFunctionType.Exp,
                    bias=mu1[:], scale=-1.0, accum_out=Zpart[:, 0:1],
                )
                nc.scalar.activation(
                    Pm1[:, P:C], pe1, mybir.ActivationFunctionType.Exp,
                    bias=mu1[:], scale=-1.0, accum_out=Zpart[:, 1:2],
                )
                nc.vector.tensor_add(
                    Zs[:, 1:2], Zpart[:, 0:1], Zpart[:, 1:2]
                )
                nc.vector.tensor_copy(Zb[:], Zs[:])
                nc.vector.reciprocal(rZ[:], Zb[:])
                for m, Pm in ((0, Pm0), (1, Pm1)):
                    for k in range(2):
                        pt = psum_t.tile(
                            [P, P], mybir.dt.float32, name="pt", tag="pt"
                        )
                        nc.tensor.transpose(pt[:], Pm[:, ts(k, P)], ident)
                        # A''T[j in k-block, i in m-block] = gamma * P^T
                        # (on DVE: cheaper psum->sbuf than ACT, and keeps
                        # the ACT queue free for exps + scales)
                        if variant.get("mul_dve", True):
                            nc.vector.tensor_scalar_mul(
                                At[:, k, ts(m, P)], pt[:], gv
                            )
                        else:
                            nc.scalar.mul(At[:, k, ts(m, P)], pt[:], gv)
                    # diagonal: += diag(Z) (falls in the k == m block)
                    dg = small.tile([P, P], f16, tag="diag")
                    nc.vector.tensor_scalar_mul(dg[:], ident, Zs[:, m : m + 1])
                    nc.vector.tensor_add(
                        At[:, m, ts(m, P)], At[:, m, ts(m, P)], dg[:]
                    )

                # software-pipeline the PE: run the PREVIOUS batch's matmul2
                # after this batch's matmul1, hiding the A'' build latency.
                this = (b, At, rZ, xn)
                todo = [pending] if pending is not None else []
                if b == NB - 1:
                    todo.append(this)
                    pending = None
                else:
                    pending = this
                for pb, pAt, prZ, pxn in todo:
                    last = pb == NB - 1
                    for m in range(2):
                        for g in range(TC // mm2_run):
                            # ot staged per group: store as soon as the
                            # group's chunks are scaled
                            ot = out_pool.tile(
                                [P, 512 * mm2_run], out_dt, tag="ot", name="ot"
                            )
                            pos = [
                                psum_o.tile(
                                    [P, 512], mybir.dt.float32,
                                    name=f"po{j}", tag="po",
                                )
                                for j in range(mm2_run)
                            ]
                            # k outer, chunk inner: mm2_run consecutive
                            # matmuls share lhsT -> walrus elides the
                            # redundant embedded weight loads
                            for k in range(2):
                                for j in range(mm2_run):
                                    nc.tensor.matmul(
                                        pos[j][:],
                                        lhsT=pAt[:, k, ts(m, P)],
                                        rhs=pxn[
                                            :, k,
                                            ts(g * mm2_run + j, 512),
                                        ],
                                        start=(k == 0),
                                        stop=(k == 1),
                                    )
                            for j in range(mm2_run):
                                # out = psum * (1/Z); alternate engines so a
                                # group's scales finish before the next
                                # group's matmuls need the banks back
                                if j % 2 == 0:
                                    nc.vector.tensor_scalar_mul(
                                        ot[:, ts(j, 512)], pos[j][:],
                                        prZ[:, m : m + 1],
                                    )
                                else:
                                    nc.scalar.mul(
                                        ot[:, ts(j, 512)], pos[j][:],
                                        prZ[:, m : m + 1],
                                    )
                            gw = 512 * mm2_run
                            o_r = o_ap[pb].rearrange(
                                "(m p) t -> p m t", p=P
                            )
                            # last batch: loads are done, so the sync HWDGE
                            # ring is idle; store per 512-chunk so the final
                            # store only waits on the last scale op
                            seng = nc.sync if last else store_eng
                            nsub = mm2_run if last else 1
                            for sh in range(nsub):
                                sw = gw // nsub
                                seng.dma_start(
                                    o_r[:, m, g * gw + sh * sw : g * gw + (sh + 1) * sw],
                                    ot[:, sh * sw : (sh + 1) * sw],
                                )

    nc.compile()
    return nc


def _get_nc(variant=None):
    key = tuple(sorted((variant or {}).items()))
    if key not in _CACHE:
        _CACHE[key] = _build_nc(variant)
    return _CACHE[key]


def _make_aux(gamma_val):
    aux = np.zeros((P, 132), dtype=np.float32)
    aux[:, 0] = gamma_val
    aux[:, 1] = 1.0 / gamma_val if gamma_val != 0 else 0.0
    aux[:, 4:132] = np.eye(P, dtype=np.float32)
    return aux


def kernel(x, gamma, _trace=False, _variant=None):
    import concourse.bass_utils as bass_utils

    variant = _variant or {}
    x = np.ascontiguousarray(np.asarray(x, dtype=np.float32))
    gamma = np.asarray(gamma, dtype=np.float32).reshape(-1)

    nc = _get_nc(variant)

    xt_np_dt = np.float32 if variant.get("xt_f32") else np.float16
    aux = _make_aux(gamma[0])
    in_maps = []
    for d in range(NCORES):
        xs = x[d * NB : (d + 1) * NB]
        in_maps.append(
            {
                "xt": np.ascontiguousarray(
                    xs.transpose(0, 2, 1).astype(xt_np_dt)
                ),
                "xn": xs.astype(np.float16),
                "aux": aux,
            }
        )

    res = bass_utils.run_bass_kernel_spmd(
        nc, in_maps, core_ids=list(range(NCORES)), trace=_trace
    )
    out = np.concatenate(
        [r["o"].astype(np.float32) for r in res.results], axis=0
    )
    if _trace:
        _CACHE["last_results"] = res
    return out
